# revision 34
# baseline (speedup 1.0000x reference)
"""Causal single-head attention (B=4, S=2048, E=1024, D=64) on 8 TRN2 NeuronCores.

Sharding: core c -> batch b = c//2, parity h = c%2. Owned query blocks are the
8 128-token blocks of parity (1-h) (h=0 -> odd, h=1 -> even), balancing causal
work 68/136 per core. No collectives: each core projects full K/V for its batch.

Perf structure: per-matmul fixed overhead (~190ns) dominates small-N
instructions, so everything is restructured into few, wide (N>=512) matmuls,
and the schedule is arranged so the PE never waits:
  - x columns are PERMUTED on the host to [owned blocks asc | other blocks asc]
    so the owned-Q projection is contiguous and scores/PV address key blocks by
    position with a graph identical across cores (SPMD); all per-core variation
    is in input data (x permutation, mab mask).
  - K and V projections are PACKED into one M=128 matmul per (col group,
    E-chunk): lhsT = [Wk_e | Wv_e], psum rows 0:64 = K^T, 64:128 = V^T.
  - phase 1 interleaves three accumulations (KV g0, KV g2, Q over both halves)
    per x chunk so compute tracks DMA arrival; constants ride in a blob DMA;
    x chunks are spread over three DMA issue queues (sync/scalar/gpsimd).
  - attention runs in 2 super-groups of 4 query blocks: group A (owned 0..3)
    over 8 key positions, group B (owned 4..7) over all 16. Below-diagonal
    query sub-blocks are not computed (shrinking-N), the boundary sub-block
    gets one [128,128] mask multiply (tri for own-parity keys, all-0/1 "mab"
    for other-parity keys). Score pairs share one bf16 psum tile and ONE exp
    instruction; pairs run one step ahead of PV so exp latency is hidden.
    Group A overlaps the remaining projections; both groups stream their
    epilogue per sub-block as soon as that sub-block's accumulation finishes.
  - scoresT layout [k, q] everywhere; PV accumulates out^T[65, q] f32 with a
    ones column appended to V (row 64 = softmax denominator); bf16 PE
    transpose + reciprocal-multiply normalizes.
"""

import itertools
import os
import sys

sys.path.insert(0, "/opt/trn_rl_repo")

import numpy as np

B, S, E, D = 4, 2048, 1024, 64
NB = S // 128      # 16 token blocks
NSLOT = NB // 2    # 8 owned query blocks per core
NE = E // 128      # 8 contraction chunks
NCORES = 8

JLIST_A = [0, 1, 2, 3, 8, 9, 10, 11]
# const blob layout (columns, bf16)
CBF_WKV, CBF_WQ, CBF_TRI, CBF_MAB, CBF_IDB = 0, 1024, 1536, 1664, 1792
CBF_N = 1920
CF_BKV, CF_BQ = 0, 1
CF_N = 2

_BUILT = {}
LAST = None  # BassKernelResults of the most recent run (for test harness)


def _build():
    variant = os.environ.get("KVARIANT", "full")
    from concourse import bacc, bass, tile, mybir

    f32 = mybir.dt.float32
    bf16 = mybir.dt.bfloat16
    MUL = mybir.AluOpType.mult
    ADD = mybir.AluOpType.add
    EXP = mybir.ActivationFunctionType.Exp

    # xbar DMA transposes verify in isolation but race with their consumers
    # under load on HW (inf outputs); keep PE transposes unless experimenting.
    dmt = variant == "dmt"

    nc = bacc.Bacc(None, target_bir_lowering=False, debug=False)

    xT_d = nc.declare_dram_parameter("xT", [128, NE * S], bf16, isOutput=False)
    cbf_d = nc.declare_dram_parameter("cbf", [128, CBF_N], bf16, isOutput=False)
    cf_d = nc.declare_dram_parameter("cf", [128, CF_N], f32, isOutput=False)
    out_d = nc.declare_dram_parameter("out", [NSLOT, 128, D], f32, isOutput=True)

    with tile.TileContext(nc) as tc:
        with (
            tc.tile_pool(name="consts", bufs=1) as consts,
            tc.tile_pool(name="xpool", bufs=NE) as xpool,
            tc.tile_pool(name="acts", bufs=1) as acts,
            tc.tile_pool(name="probs", bufs=4) as probs,
            tc.tile_pool(name="smalls", bufs=3) as smalls,
            tc.tile_pool(name="ps_p", bufs=2, space="PSUM") as ps_p,
            tc.tile_pool(name="ps_q", bufs=1, space="PSUM") as ps_q,
            tc.tile_pool(name="ps_sc", bufs=3 if dmt else 2, space="PSUM") as ps_sc,
            tc.tile_pool(name="ps_o", bufs=2 if dmt else 1, space="PSUM") as ps_o,
            tc.tile_pool(name="ps_t", bufs=2, space="PSUM") as ps_t,
            # banks (dmt): ps_p 2 + ps_q 1 + ps_sc 3 + ps_o 2 = 8 (ps_t unused)
            # banks (pet): ps_p 2 + ps_q 1 + ps_sc 2 + ps_o 1 + ps_t 2 = 8
        ):
            # ---- constants: blob DMAs
            cbf = consts.tile([128, CBF_N], bf16, tag="cbf")
            cf = consts.tile([128, CF_N], f32, tag="cf")
            nc.sync.dma_start(cbf[:], cbf_d[:])
            nc.scalar.dma_start(cf[:], cf_d[:])
            wkv = cbf[:, CBF_WKV : CBF_WKV + NE * 128]
            wq = cbf[:, CBF_WQ : CBF_WQ + NE * D]
            tri = cbf[:, CBF_TRI : CBF_TRI + 128]
            mab = cbf[:, CBF_MAB : CBF_MAB + 128]
            idb = cbf[:, CBF_IDB : CBF_IDB + 128]
            bkv = cf[:, CF_BKV : CF_BKV + 1]
            bq = cf[0:D, CF_BQ : CF_BQ + 1]

            # ---- x^T tiles (permuted cols), spread over three issue queues
            xt = []
            qeng = [nc.sync, nc.scalar, nc.gpsimd, nc.sync,
                    nc.scalar, nc.gpsimd, nc.sync, nc.scalar]
            for e in range(NE):
                t = xpool.tile([128, S], bf16, tag="xt")
                qeng[e].dma_start(t[:], xT_d[:, e * S : (e + 1) * S])
                xt.append(t)

            # ---- persistent activations
            kv_sb = acts.tile([128, S], bf16, tag="kv")     # 0:64 K^T, 64:128 V^T
            qown = acts.tile([D, NSLOT * 128], bf16, tag="qown")
            vsb = acts.tile([128, NB, D + 1], bf16, tag="vsb")
            nc.vector.memset(vsb[:, :, D : D + 1], 1.0)

            def vt_one(j):
                # V natural block j: transpose the V rows of the packed K/V
                # slice (xbar DMA, or bf16 PE transpose as fallback).
                if dmt:
                    nc.sync.dma_start_transpose(
                        vsb[:, j, 0:D], kv_sb[D:128, j * 128 : (j + 1) * 128]
                    )
                    return
                pv = ps_t.tile([128, 130], bf16, tag="ps_vt")
                nc.tensor.transpose(
                    pv[:, 0:128], kv_sb[:, j * 128 : (j + 1) * 128], idb
                )
                yield
                nc.vector.tensor_copy(vsb[:, j, 0:D], pv[:, 64:128])

            def kv_finish(g, p):
                cols = slice(g * 512, (g + 1) * 512)
                nc.vector.tensor_scalar(kv_sb[:, cols], p[:], bkv, None, ADD)

            def kv_group(g):
                cols = slice(g * 512, (g + 1) * 512)
                p = ps_p.tile([128, 512], f32, tag="pp")
                for e in range(NE):
                    nc.tensor.matmul(
                        p[:],
                        wkv[:, e * 128 : (e + 1) * 128],
                        xt[e][:, cols],
                        start=(e == 0),
                        stop=(e == NE - 1),
                    )
                    yield
                kv_finish(g, p)
                for j in range(4 * g, 4 * g + 4):
                    yield from vt_one(j)

            def q_group(g):
                cols = slice(g * 512, (g + 1) * 512)
                p = ps_q.tile([D, 512], f32, tag="ppq")
                for e in range(NE):
                    nc.tensor.matmul(
                        p[:],
                        wq[:, e * D : (e + 1) * D],
                        xt[e][:, cols],
                        start=(e == 0),
                        stop=(e == NE - 1),
                    )
                    yield
                nc.vector.tensor_scalar(qown[:, cols], p[:], bq, None, ADD)

            def attn(jlist, spec, qbase, pout, slot_base, epi_from):
                n = len(jlist)
                pts = {}

                def emit_score(idx):
                    nz, mt = spec[idx]
                    j = jlist[idx]
                    qc = slice(nz * 128, 512)
                    psc = ps_sc.tile([128, 512], f32, tag="psc")
                    nc.tensor.matmul(
                        psc[:, qc],
                        kv_sb[0:D, j * 128 : (j + 1) * 128],
                        qown[:, qbase + nz * 128 : qbase + 512],
                        start=True,
                        stop=True,
                    )
                    pt = probs.tile([128, 512], bf16, tag="pt")
                    nc.scalar.activation(pt[:, qc], psc[:, qc], EXP)
                    if mt is not None:
                        mc = slice(nz * 128, (nz + 1) * 128)
                        nc.vector.tensor_mul(pt[:, mc], pt[:, mc], mt)
                    pts[idx] = (pt, qc)

                depth = 2 if dmt else 1
                for i in range(depth):
                    emit_score(i)
                    yield
                for idx in range(n):
                    if idx + depth < n:
                        emit_score(idx + depth)
                        yield
                    pt, qc = pts.pop(idx)
                    # skip_group_check: the streamed epilogue reads finished
                    # sub-block columns while the bank's accumulation group is
                    # still open for higher columns (fine on HW, sim-only
                    # check)
                    nc.tensor.matmul(
                        pout[:, qc],
                        vsb[:, jlist[idx], :],
                        pt[:, qc],
                        start=(idx == 0),
                        stop=(idx == n - 1),
                        skip_group_check=True,
                    )
                    yield
                    # stream the epilogue: sub-block s is final after the PV
                    # at idx == epi_from + s
                    s = idx - epi_from
                    if 0 <= s < 4:
                        if dmt:
                            # bf16 copy (rows 0:65 of an 80-row tile so the
                            # xbar sees whole 16-row tiles; rows 65:80 are
                            # garbage that lands in unread dst columns)
                            ot = smalls.tile([80, 128], bf16, tag="otT")
                            nc.gpsimd.memset(ot[D : 80, :], 0.0)
                            nc.vector.tensor_copy(
                                ot[0 : D + 1, :], pout[:, s * 128 : (s + 1) * 128]
                            )
                            nat = smalls.tile([128, 80], bf16, tag="nat")
                            nc.sync.dma_start_transpose(nat[:], ot[:])
                            rcp = smalls.tile([128, 1], f32, tag="rcp")
                            nc.vector.reciprocal(rcp[:], nat[:, D : D + 1])
                            fin = smalls.tile([128, D], f32, tag="fin")
                            nc.vector.tensor_scalar(
                                fin[:], nat[:, 0:D], rcp[:], None, MUL
                            )
                            nc.sync.dma_start(out_d[slot_base + s], fin[:])
                            continue
                        ot = smalls.tile([D + 1, 128], bf16, tag="otT")
                        nc.vector.tensor_copy(
                            ot[:], pout[:, s * 128 : (s + 1) * 128]
                        )
                        ptr = ps_t.tile([128, 130], bf16, tag="ps_vt")
                        nc.tensor.transpose(
                            ptr[:, 0 : D + 1], ot[:], idb[0 : D + 1, 0 : D + 1]
                        )
                        yield
                        rcp = smalls.tile([128, 1], f32, tag="rcp")
                        nc.vector.reciprocal(rcp[:], ptr[:, D : D + 1])
                        fin = smalls.tile([128, D], f32, tag="fin")
                        nc.vector.tensor_scalar(
                            fin[:], ptr[:, 0:D], rcp[:], None, MUL
                        )
                        nc.sync.dma_start(out_d[slot_base + s], fin[:])

            # group A: key positions 0..3 (own-parity, tri on diag) and 8..11
            # (other-parity, mab on boundary); below-diagonal sub-blocks are
            # skipped via shrinking-N.
            spec_a = [(i, tri) for i in range(4)] + [(i, mab) for i in range(4)]
            # group B: owned blocks 4..7 vs all 16 key positions.
            spec_b = []
            for j in range(16):
                if 4 <= j < 8:
                    spec_b.append((j - 4, tri))
                elif 12 <= j:
                    spec_b.append((j - 12, mab))
                else:
                    spec_b.append((0, None))

            def run(gen):
                for _ in gen:
                    pass

            # ---- phase 1: KV g0, KV g2 and Q accumulate together, chunk by
            # chunk, tracking x DMA arrival
            pg0 = ps_p.tile([128, 512], f32, tag="pp")
            pg2 = ps_p.tile([128, 512], f32, tag="pp")
            qa = q_group(0)
            for e in range(NE):
                nc.tensor.matmul(
                    pg0[:], wkv[:, e * 128 : (e + 1) * 128], xt[e][:, 0:512],
                    start=(e == 0), stop=(e == NE - 1),
                )
                nc.tensor.matmul(
                    pg2[:], wkv[:, e * 128 : (e + 1) * 128], xt[e][:, 1024:1536],
                    start=(e == 0), stop=(e == NE - 1),
                )
                next(qa, None)
            next(qa, None)  # emit the q bias-add
            kv_finish(0, pg0)
            kv_finish(2, pg2)
            for j in JLIST_A:
                run(vt_one(j))

            # ---- phase 2: attn A, interleaving the remaining projections
            poutA = ps_o.tile([D + 1, 512], f32, tag="pout")
            fillers = itertools.chain(kv_group(1), kv_group(3), q_group(1))
            for _ in attn(JLIST_A, spec_a, 0, poutA, 0, 4):
                next(fillers, None)
                next(fillers, None)
            for _ in fillers:
                pass

            # ---- phase 3: attn B with streamed epilogue
            poutB = ps_o.tile([D + 1, 512], f32, tag="pout")
            run(attn(list(range(16)), spec_b, 512, poutB, 4, 12))

    nc.compile()
    return nc


def _get_nc():
    key = os.environ.get("KVARIANT", "full")
    if key not in _BUILT:
        _BUILT[key] = _build()
    return _BUILT[key]


def _host_inputs(x, Wq, bq, Wk, bk, Wv, bv):
    """Build the 8 per-core input maps."""
    import ml_dtypes

    bf = ml_dtypes.bfloat16
    x = np.asarray(x, np.float32)
    cbf0 = np.zeros((128, CBF_N), np.float32)
    cbf0[:, CBF_WKV : CBF_WKV + NE * 128] = (
        np.concatenate(
            [
                np.asarray(Wk, np.float32).reshape(NE, 128, D),
                np.asarray(Wv, np.float32).reshape(NE, 128, D),
            ],
            axis=2,
        )
        .transpose(1, 0, 2)
        .reshape(128, NE * 128)
    )
    cbf0[:, CBF_WQ : CBF_WQ + NE * D] = (
        (np.asarray(Wq, np.float32) / float(D))
        .reshape(NE, 128, D)
        .transpose(1, 0, 2)
        .reshape(128, NE * D)
    )
    cbf0[:, CBF_TRI : CBF_TRI + 128] = np.triu(np.ones((128, 128), np.float32))
    cbf0[:, CBF_IDB : CBF_IDB + 128] = np.eye(128, dtype=np.float32)

    cf = np.zeros((128, CF_N), np.float32)
    cf[:, CF_BKV] = np.concatenate(
        [np.asarray(bk, np.float32), np.asarray(bv, np.float32)]
    )
    cf[0:D, CF_BQ] = np.asarray(bq, np.float32) / float(D)

    xbT = [np.ascontiguousarray(x[b].T) for b in range(B)]  # [E, S]
    in_maps = []
    for c in range(NCORES):
        b, h = c // 2, c % 2
        perm = [2 * p + (1 - h) for p in range(8)] + [2 * p + h for p in range(8)]
        xp = xbT[b].reshape(E, NB, 128)[:, perm, :].reshape(E, S)
        xT = (
            xp.reshape(NE, 128, S).transpose(1, 0, 2).reshape(128, NE * S).astype(bf)
        )
        cbf = cbf0.copy()
        cbf[:, CBF_MAB : CBF_MAB + 128] = 1.0 - h
        in_maps.append({
            "xT": xT,
            "cbf": cbf.astype(bf),
            "cf": cf,
        })
    return in_maps


def _assemble(results):
    out = np.zeros((B, S, D), np.float32)
    for c in range(NCORES):
        b, h = c // 2, c % 2
        o = np.asarray(results[c]["out"]).reshape(NSLOT, 128, D)
        for i in range(NSLOT):
            g = 2 * i + (1 - h)
            out[b, g * 128 : (g + 1) * 128] = o[i]
    return out


def kernel(x, Wq, bq, Wk, bk, Wv, bv):
    global LAST
    from concourse.bass_utils import run_bass_kernel_spmd

    nc = _get_nc()
    in_maps = _host_inputs(x, Wq, bq, Wk, bk, Wv, bv)
    LAST = run_bass_kernel_spmd(nc, in_maps, list(range(NCORES)))
    return _assemble(LAST.results)


# revision 40
# speedup vs baseline: 1.0731x; 1.0731x over previous
"""Causal single-head attention (B=4, S=2048, E=1024, D=64) on 8 TRN2 NeuronCores.

Sharding: core c -> batch b = c//2, parity h = c%2. Owned query blocks are the
8 128-token blocks of parity (1-h) (h=0 -> odd, h=1 -> even), balancing causal
work 68/136 per core. No collectives: each core projects full K/V for its batch.

Perf structure: per-matmul fixed overhead (~190ns) dominates small-N
instructions, so everything is restructured into few, wide (N>=512) matmuls,
and the schedule is arranged so the PE never waits:
  - x columns are PERMUTED on the host to [owned blocks asc | other blocks asc]
    so the owned-Q projection is contiguous and scores/PV address key blocks by
    position with a graph identical across cores (SPMD); all per-core variation
    is in input data (x permutation, mab mask).
  - K and V projections are PACKED into one M=128 matmul per (col group,
    E-chunk): lhsT = [Wk_e | Wv_e], psum rows 0:64 = K^T, 64:128 = V^T.
  - phase 1 interleaves three accumulations (KV g0, KV g2, Q over both halves)
    per x chunk so compute tracks DMA arrival; constants ride in a blob DMA;
    x chunks are spread over three DMA issue queues (sync/scalar/gpsimd).
  - attention runs in 2 super-groups of 4 query blocks: group A (owned 0..3)
    over 8 key positions, group B (owned 4..7) over all 16. Below-diagonal
    query sub-blocks are not computed (shrinking-N), the boundary sub-block
    gets one [128,128] mask multiply (tri for own-parity keys, all-0/1 "mab"
    for other-parity keys). Score pairs share one bf16 psum tile and ONE exp
    instruction; pairs run one step ahead of PV so exp latency is hidden.
    Group A overlaps the remaining projections; both groups stream their
    epilogue per sub-block as soon as that sub-block's accumulation finishes.
  - scoresT layout [k, q] everywhere; PV accumulates out^T[65, q] f32 with a
    ones column appended to V (row 64 = softmax denominator); bf16 PE
    transpose + reciprocal-multiply normalizes.
"""

import itertools
import os
import sys

sys.path.insert(0, "/opt/trn_rl_repo")

import numpy as np

B, S, E, D = 4, 2048, 1024, 64
NB = S // 128      # 16 token blocks
NSLOT = NB // 2    # 8 owned query blocks per core
NE = E // 128      # 8 contraction chunks
NCORES = 8

JLIST_A = [0, 1, 2, 3, 8, 9, 10, 11]
# const blob layout (columns, bf16)
CBF_WKV, CBF_WQ, CBF_TRI, CBF_MAB, CBF_IDB = 0, 1024, 1536, 1664, 1792
CBF_N = 1920
CF_BKV, CF_BQ = 0, 1
CF_N = 2

_BUILT = {}
LAST = None  # BassKernelResults of the most recent run (for test harness)


def _build():
    variant = os.environ.get("KVARIANT", "full")
    from concourse import bacc, bass, tile, mybir

    f32 = mybir.dt.float32
    bf16 = mybir.dt.bfloat16
    MUL = mybir.AluOpType.mult
    ADD = mybir.AluOpType.add
    EXP = mybir.ActivationFunctionType.Exp

    # xbar DMA transposes verify in isolation but race with their consumers
    # under load on HW (inf outputs); keep PE transposes unless experimenting.
    dmt = variant == "dmt"

    nc = bacc.Bacc(None, target_bir_lowering=False, debug=False)

    xT_d = nc.declare_dram_parameter("xT", [128, NE * S], bf16, isOutput=False)
    cbf_d = nc.declare_dram_parameter("cbf", [128, CBF_N], bf16, isOutput=False)
    cf_d = nc.declare_dram_parameter("cf", [128, CF_N], f32, isOutput=False)
    out_d = nc.declare_dram_parameter("out", [NSLOT, 128, D], f32, isOutput=True)

    with tile.TileContext(nc) as tc:
        with (
            tc.tile_pool(name="consts", bufs=1) as consts,
            tc.tile_pool(name="xpool", bufs=NE) as xpool,
            tc.tile_pool(name="acts", bufs=1) as acts,
            tc.tile_pool(name="probs", bufs=7) as probs,
            tc.tile_pool(name="smalls", bufs=3) as smalls,
            tc.tile_pool(name="ps_p", bufs=2, space="PSUM") as ps_p,
            tc.tile_pool(name="ps_q", bufs=1, space="PSUM") as ps_q,
            tc.tile_pool(name="ps_sc", bufs=3 if dmt else 2, space="PSUM") as ps_sc,
            tc.tile_pool(name="ps_o", bufs=2 if dmt else 1, space="PSUM") as ps_o,
            tc.tile_pool(name="ps_t", bufs=2, space="PSUM") as ps_t,
            # banks (dmt): ps_p 2 + ps_q 1 + ps_sc 3 + ps_o 2 = 8 (ps_t unused)
            # banks (pet): ps_p 2 + ps_q 1 + ps_sc 2 + ps_o 1 + ps_t 2 = 8
        ):
            # ---- constants: blob DMAs
            cbf = consts.tile([128, CBF_N], bf16, tag="cbf")
            cf = consts.tile([128, CF_N], f32, tag="cf")
            nc.sync.dma_start(cbf[:], cbf_d[:])
            nc.scalar.dma_start(cf[:], cf_d[:])
            wkv = cbf[:, CBF_WKV : CBF_WKV + NE * 128]
            wq = cbf[:, CBF_WQ : CBF_WQ + NE * D]
            tri = cbf[:, CBF_TRI : CBF_TRI + 128]
            mab = cbf[:, CBF_MAB : CBF_MAB + 128]
            idb = cbf[:, CBF_IDB : CBF_IDB + 128]
            bkv = cf[:, CF_BKV : CF_BKV + 1]
            bq = cf[0:D, CF_BQ : CF_BQ + 1]

            # ---- x^T tiles (permuted cols), spread over three issue queues
            xt = []
            qeng = [nc.sync, nc.scalar, nc.gpsimd, nc.sync,
                    nc.scalar, nc.gpsimd, nc.sync, nc.scalar]
            for e in range(NE):
                t = xpool.tile([128, S], bf16, tag="xt")
                qeng[e].dma_start(t[:], xT_d[:, e * S : (e + 1) * S])
                xt.append(t)

            # ---- persistent activations
            kv_sb = acts.tile([128, S], bf16, tag="kv")     # 0:64 K^T, 64:128 V^T
            qown = acts.tile([D, NSLOT * 128], bf16, tag="qown")
            vsb = acts.tile([128, NB, D + 1], bf16, tag="vsb")
            nc.vector.memset(vsb[:, :, D : D + 1], 1.0)

            def vt_one(j):
                # V natural block j: transpose the V rows of the packed K/V
                # slice (xbar DMA, or bf16 PE transpose as fallback).
                if dmt:
                    nc.sync.dma_start_transpose(
                        vsb[:, j, 0:D], kv_sb[D:128, j * 128 : (j + 1) * 128]
                    )
                    return
                pv = ps_t.tile([128, 130], bf16, tag="ps_vt")
                nc.tensor.transpose(
                    pv[:, 0:128], kv_sb[:, j * 128 : (j + 1) * 128], idb
                )
                yield
                nc.vector.tensor_copy(vsb[:, j, 0:D], pv[:, 64:128])

            def kv_finish(g, p):
                cols = slice(g * 512, (g + 1) * 512)
                nc.vector.tensor_scalar(kv_sb[:, cols], p[:], bkv, None, ADD)

            def kv_group(g):
                cols = slice(g * 512, (g + 1) * 512)
                p = ps_p.tile([128, 512], f32, tag="pp")
                for e in range(NE):
                    nc.tensor.matmul(
                        p[:],
                        wkv[:, e * 128 : (e + 1) * 128],
                        xt[e][:, cols],
                        start=(e == 0),
                        stop=(e == NE - 1),
                    )
                    yield
                kv_finish(g, p)
                for j in range(4 * g, 4 * g + 4):
                    yield from vt_one(j)

            def q_group(g):
                cols = slice(g * 512, (g + 1) * 512)
                p = ps_q.tile([D, 512], f32, tag="ppq")
                for e in range(NE):
                    nc.tensor.matmul(
                        p[:],
                        wq[:, e * D : (e + 1) * D],
                        xt[e][:, cols],
                        start=(e == 0),
                        stop=(e == NE - 1),
                    )
                    yield
                nc.vector.tensor_scalar(qown[:, cols], p[:], bq, None, ADD)

            def mk_state(jlist, spec, qbase, pool, tag):
                return {"jlist": jlist, "spec": spec, "qbase": qbase,
                        "pool": pool, "tag": tag, "pts": {}}

            def emit_score(st, idx):
                nz, mt = st["spec"][idx]
                j = st["jlist"][idx]
                qbase = st["qbase"]
                qc = slice(nz * 128, 512)
                psc = st["pool"].tile([128, 512], f32, tag=st["tag"])
                nc.tensor.matmul(
                    psc[:, qc],
                    kv_sb[0:D, j * 128 : (j + 1) * 128],
                    qown[:, qbase + nz * 128 : qbase + 512],
                    start=True,
                    stop=True,
                )
                pt = probs.tile([128, 512], bf16, tag="pt")
                nc.scalar.activation(pt[:, qc], psc[:, qc], EXP)
                if mt is not None:
                    mc = slice(nz * 128, (nz + 1) * 128)
                    nc.vector.tensor_mul(pt[:, mc], pt[:, mc], mt)
                st["pts"][idx] = (pt, qc)

            def attn_scores(st, idxs):
                for idx in idxs:
                    emit_score(st, idx)
                    yield

            def attn(st, pout, slot_base, epi_from, pre=0):
                # PVs for all idx; scores emitted one ahead, except the first
                # `pre` which were pre-emitted elsewhere (attn_scores)
                jlist, pts = st["jlist"], st["pts"]
                n = len(jlist)
                depth = 2 if dmt else 1
                for i in range(min(depth, n)):
                    if i >= pre:
                        emit_score(st, i)
                        yield
                for idx in range(n):
                    t = idx + depth
                    if t < n and t >= pre:
                        emit_score(st, t)
                        yield
                    pt, qc = pts.pop(idx)
                    # skip_group_check: the streamed epilogue reads finished
                    # sub-block columns while the bank's accumulation group is
                    # still open for higher columns (fine on HW, sim-only
                    # check)
                    nc.tensor.matmul(
                        pout[:, qc],
                        vsb[:, jlist[idx], :],
                        pt[:, qc],
                        start=(idx == 0),
                        stop=(idx == n - 1),
                        skip_group_check=True,
                    )
                    yield
                    # stream the epilogue: sub-block s is final after the PV
                    # at idx == epi_from + s
                    s = idx - epi_from
                    if 0 <= s < 4:
                        if dmt:
                            # bf16 copy (rows 0:65 of an 80-row tile so the
                            # xbar sees whole 16-row tiles; rows 65:80 are
                            # garbage that lands in unread dst columns)
                            ot = smalls.tile([80, 128], bf16, tag="otT")
                            nc.gpsimd.memset(ot[D : 80, :], 0.0)
                            nc.vector.tensor_copy(
                                ot[0 : D + 1, :], pout[:, s * 128 : (s + 1) * 128]
                            )
                            nat = smalls.tile([128, 80], bf16, tag="nat")
                            nc.sync.dma_start_transpose(nat[:], ot[:])
                            rcp = smalls.tile([128, 1], f32, tag="rcp")
                            nc.vector.reciprocal(rcp[:], nat[:, D : D + 1])
                            fin = smalls.tile([128, D], f32, tag="fin")
                            nc.vector.tensor_scalar(
                                fin[:], nat[:, 0:D], rcp[:], None, MUL
                            )
                            nc.sync.dma_start(out_d[slot_base + s], fin[:])
                            continue
                        ot = smalls.tile([D + 1, 128], bf16, tag="otT")
                        nc.vector.tensor_copy(
                            ot[:], pout[:, s * 128 : (s + 1) * 128]
                        )
                        ptr = ps_t.tile([128, 130], bf16, tag="ps_vt")
                        nc.tensor.transpose(
                            ptr[:, 0 : D + 1], ot[:], idb[0 : D + 1, 0 : D + 1]
                        )
                        yield
                        rcp = smalls.tile([128, 1], f32, tag="rcp")
                        nc.vector.reciprocal(rcp[:], ptr[:, D : D + 1])
                        fin = smalls.tile([128, D], f32, tag="fin")
                        nc.vector.tensor_scalar(
                            fin[:], ptr[:, 0:D], rcp[:], None, MUL
                        )
                        nc.sync.dma_start(out_d[slot_base + s], fin[:])

            # group A: key positions 0..3 (own-parity, tri on diag) and 8..11
            # (other-parity, mab on boundary); below-diagonal sub-blocks are
            # skipped via shrinking-N.
            spec_a = [(i, tri) for i in range(4)] + [(i, mab) for i in range(4)]
            # group B: owned blocks 4..7 vs all 16 key positions.
            spec_b = []
            for j in range(16):
                if 4 <= j < 8:
                    spec_b.append((j - 4, tri))
                elif 12 <= j:
                    spec_b.append((j - 12, mab))
                else:
                    spec_b.append((0, None))

            def run(gen):
                for _ in gen:
                    pass

            # ---- phase 1: KV g0, KV g2 and Q accumulate together, chunk by
            # chunk, tracking x DMA arrival
            pg0 = ps_p.tile([128, 512], f32, tag="pp")
            pg2 = ps_p.tile([128, 512], f32, tag="pp")
            qa = q_group(0)
            for e in range(NE):
                nc.tensor.matmul(
                    pg0[:], wkv[:, e * 128 : (e + 1) * 128], xt[e][:, 0:512],
                    start=(e == 0), stop=(e == NE - 1),
                )
                nc.tensor.matmul(
                    pg2[:], wkv[:, e * 128 : (e + 1) * 128], xt[e][:, 1024:1536],
                    start=(e == 0), stop=(e == NE - 1),
                )
                next(qa, None)
            next(qa, None)  # emit the q bias-add
            kv_finish(0, pg0)
            kv_finish(2, pg2)
            for j in JLIST_A:
                run(vt_one(j))

            # ---- phase 2: attn A, interleaving the remaining projections
            stA = mk_state(JLIST_A, spec_a, 0, ps_sc, "psc")
            stB = mk_state(list(range(16)), spec_b, 512, ps_p, "pp")
            poutA = ps_o.tile([D + 1, 512], f32, tag="pout")
            # attn B's first 4 (maskless) scores+exps ride in phase 2 where
            # the scalar engine has slack; their PVs run in phase 3
            fillers = itertools.chain(
                q_group(1),
                attn_scores(stB, range(0, 4)),
                kv_group(1),
                kv_group(3),
            )
            for _ in attn(stA, poutA, 0, 4):
                next(fillers, None)
                next(fillers, None)
            for _ in fillers:
                pass

            # ---- phase 3: attn B with streamed epilogue
            poutB = ps_o.tile([D + 1, 512], f32, tag="pout")
            run(attn(stB, poutB, 4, 12, pre=4))

    nc.compile()
    return nc


def _get_nc():
    key = os.environ.get("KVARIANT", "full")
    if key not in _BUILT:
        _BUILT[key] = _build()
    return _BUILT[key]


def _host_inputs(x, Wq, bq, Wk, bk, Wv, bv):
    """Build the 8 per-core input maps."""
    import ml_dtypes

    bf = ml_dtypes.bfloat16
    x = np.asarray(x, np.float32)
    cbf0 = np.zeros((128, CBF_N), np.float32)
    cbf0[:, CBF_WKV : CBF_WKV + NE * 128] = (
        np.concatenate(
            [
                np.asarray(Wk, np.float32).reshape(NE, 128, D),
                np.asarray(Wv, np.float32).reshape(NE, 128, D),
            ],
            axis=2,
        )
        .transpose(1, 0, 2)
        .reshape(128, NE * 128)
    )
    cbf0[:, CBF_WQ : CBF_WQ + NE * D] = (
        (np.asarray(Wq, np.float32) / float(D))
        .reshape(NE, 128, D)
        .transpose(1, 0, 2)
        .reshape(128, NE * D)
    )
    cbf0[:, CBF_TRI : CBF_TRI + 128] = np.triu(np.ones((128, 128), np.float32))
    cbf0[:, CBF_IDB : CBF_IDB + 128] = np.eye(128, dtype=np.float32)

    cf = np.zeros((128, CF_N), np.float32)
    cf[:, CF_BKV] = np.concatenate(
        [np.asarray(bk, np.float32), np.asarray(bv, np.float32)]
    )
    cf[0:D, CF_BQ] = np.asarray(bq, np.float32) / float(D)

    xbT = [np.ascontiguousarray(x[b].T) for b in range(B)]  # [E, S]
    in_maps = []
    for c in range(NCORES):
        b, h = c // 2, c % 2
        perm = [2 * p + (1 - h) for p in range(8)] + [2 * p + h for p in range(8)]
        xp = xbT[b].reshape(E, NB, 128)[:, perm, :].reshape(E, S)
        xT = (
            xp.reshape(NE, 128, S).transpose(1, 0, 2).reshape(128, NE * S).astype(bf)
        )
        cbf = cbf0.copy()
        cbf[:, CBF_MAB : CBF_MAB + 128] = 1.0 - h
        in_maps.append({
            "xT": xT,
            "cbf": cbf.astype(bf),
            "cf": cf,
        })
    return in_maps


def _assemble(results):
    out = np.zeros((B, S, D), np.float32)
    for c in range(NCORES):
        b, h = c // 2, c % 2
        o = np.asarray(results[c]["out"]).reshape(NSLOT, 128, D)
        for i in range(NSLOT):
            g = 2 * i + (1 - h)
            out[b, g * 128 : (g + 1) * 128] = o[i]
    return out


def kernel(x, Wq, bq, Wk, bk, Wv, bv):
    global LAST
    from concourse.bass_utils import run_bass_kernel_spmd

    nc = _get_nc()
    in_maps = _host_inputs(x, Wq, bq, Wk, bk, Wv, bv)
    LAST = run_bass_kernel_spmd(nc, in_maps, list(range(NCORES)))
    return _assemble(LAST.results)


# revision 44
# speedup vs baseline: 1.1293x; 1.0523x over previous
"""Causal single-head attention (B=4, S=2048, E=1024, D=64) on 8 TRN2 NeuronCores.

Sharding: core c -> batch b = c//2, parity h = c%2. Owned query blocks are the
8 128-token blocks of parity (1-h) (h=0 -> odd, h=1 -> even), balancing causal
work 68/136 per core. No collectives: each core projects full K/V for its batch.

Perf structure: per-matmul fixed overhead (~190ns) dominates small-N
instructions, so everything is restructured into few, wide (N>=512) matmuls,
and the schedule is arranged so the PE never waits:
  - x columns are PERMUTED on the host to [owned blocks asc | other blocks asc]
    so the owned-Q projection is contiguous and scores/PV address key blocks by
    position with a graph identical across cores (SPMD); all per-core variation
    is in input data (x permutation, mab mask).
  - K and V projections are PACKED into one M=128 matmul per (col group,
    E-chunk): lhsT = [Wk_e | Wv_e], psum rows 0:64 = K^T, 64:128 = V^T.
  - phase 1 interleaves three accumulations (KV g0, KV g2, Q over both halves)
    per x chunk so compute tracks DMA arrival; constants ride in a blob DMA;
    x chunks are spread over three DMA issue queues (sync/scalar/gpsimd).
  - attention runs in 2 super-groups of 4 query blocks: group A (owned 0..3)
    over 8 key positions, group B (owned 4..7) over all 16. Below-diagonal
    query sub-blocks are not computed (shrinking-N), the boundary sub-block
    gets one [128,128] mask multiply (tri for own-parity keys, all-0/1 "mab"
    for other-parity keys). Score pairs share one bf16 psum tile and ONE exp
    instruction; pairs run one step ahead of PV so exp latency is hidden.
    Group A overlaps the remaining projections; both groups stream their
    epilogue per sub-block as soon as that sub-block's accumulation finishes.
  - scoresT layout [k, q] everywhere; PV accumulates out^T[65, q] f32 with a
    ones column appended to V (row 64 = softmax denominator); bf16 PE
    transpose + reciprocal-multiply normalizes.
"""

import itertools
import os
import sys

sys.path.insert(0, "/opt/trn_rl_repo")

import numpy as np

B, S, E, D = 4, 2048, 1024, 64
NB = S // 128      # 16 token blocks
NSLOT = NB // 2    # 8 owned query blocks per core
NE = E // 128      # 8 contraction chunks
NCORES = 8

JLIST_A = [0, 1, 2, 3, 8, 9, 10, 11]
# const blob layout (columns, bf16)
CBF_WKV, CBF_WQ, CBF_TRI, CBF_MAB, CBF_IDB = 0, 1024, 1536, 1664, 1792
CBF_N = 1920
CF_BKV, CF_BQ = 0, 1
CF_N = 2

_BUILT = {}
LAST = None  # BassKernelResults of the most recent run (for test harness)


def _build():
    variant = os.environ.get("KVARIANT", "full")
    from concourse import bacc, bass, tile, mybir

    f32 = mybir.dt.float32
    bf16 = mybir.dt.bfloat16
    MUL = mybir.AluOpType.mult
    ADD = mybir.AluOpType.add
    EXP = mybir.ActivationFunctionType.Exp

    # xbar DMA transposes verify in isolation but race with their consumers
    # under load on HW (inf outputs); keep PE transposes unless experimenting.
    dmt = variant == "dmt"

    nc = bacc.Bacc(None, target_bir_lowering=False, debug=False)

    xT_d = nc.declare_dram_parameter("xT", [128, NE * S], bf16, isOutput=False)
    cbf_d = nc.declare_dram_parameter("cbf", [128, CBF_N], bf16, isOutput=False)
    cf_d = nc.declare_dram_parameter("cf", [128, CF_N], f32, isOutput=False)
    out_d = nc.declare_dram_parameter("out", [NSLOT, 128, D], f32, isOutput=True)

    with tile.TileContext(nc) as tc:
        with (
            tc.tile_pool(name="consts", bufs=1) as consts,
            tc.tile_pool(name="xpool", bufs=NE) as xpool,
            tc.tile_pool(name="acts", bufs=1) as acts,
            tc.tile_pool(name="probs", bufs=12) as probs,
            tc.tile_pool(name="smalls", bufs=3) as smalls,
            tc.tile_pool(name="ps_p", bufs=2, space="PSUM") as ps_p,
            tc.tile_pool(name="ps_q", bufs=1, space="PSUM") as ps_q,
            tc.tile_pool(name="ps_sc", bufs=3 if dmt else 2, space="PSUM") as ps_sc,
            tc.tile_pool(name="ps_o", bufs=2 if dmt else 1, space="PSUM") as ps_o,
            tc.tile_pool(name="ps_t", bufs=2, space="PSUM") as ps_t,
            # banks (dmt): ps_p 2 + ps_q 1 + ps_sc 3 + ps_o 2 = 8 (ps_t unused)
            # banks (pet): ps_p 2 + ps_q 1 + ps_sc 2 + ps_o 1 + ps_t 2 = 8
        ):
            # ---- constants: blob DMAs
            cbf = consts.tile([128, CBF_N], bf16, tag="cbf")
            cf = consts.tile([128, CF_N], f32, tag="cf")
            nc.sync.dma_start(cbf[:], cbf_d[:])
            nc.scalar.dma_start(cf[:], cf_d[:])
            wkv = cbf[:, CBF_WKV : CBF_WKV + NE * 128]
            wq = cbf[:, CBF_WQ : CBF_WQ + NE * D]
            tri = cbf[:, CBF_TRI : CBF_TRI + 128]
            mab = cbf[:, CBF_MAB : CBF_MAB + 128]
            idb = cbf[:, CBF_IDB : CBF_IDB + 128]
            bkv = cf[:, CF_BKV : CF_BKV + 1]
            bq = cf[0:D, CF_BQ : CF_BQ + 1]

            # ---- x^T tiles (permuted cols), spread over three issue queues
            xt = []
            qeng = [nc.sync, nc.scalar, nc.gpsimd, nc.sync,
                    nc.scalar, nc.gpsimd, nc.sync, nc.scalar]
            for e in range(NE):
                t = xpool.tile([128, S], bf16, tag="xt")
                qeng[e].dma_start(t[:], xT_d[:, e * S : (e + 1) * S])
                xt.append(t)

            # ---- persistent activations
            kv_sb = acts.tile([128, S], bf16, tag="kv")     # 0:64 K^T, 64:128 V^T
            qown = acts.tile([D, NSLOT * 128], bf16, tag="qown")
            vsb = acts.tile([128, NB, D + 1], bf16, tag="vsb")
            nc.vector.memset(vsb[:, :, D : D + 1], 1.0)

            def vt_one(j):
                # V natural block j: transpose the V rows of the packed K/V
                # slice (xbar DMA, or bf16 PE transpose as fallback).
                if dmt:
                    nc.sync.dma_start_transpose(
                        vsb[:, j, 0:D], kv_sb[D:128, j * 128 : (j + 1) * 128]
                    )
                    return
                pv = ps_t.tile([128, 130], bf16, tag="ps_vt")
                nc.tensor.transpose(
                    pv[:, 0:128], kv_sb[:, j * 128 : (j + 1) * 128], idb
                )
                yield
                nc.vector.tensor_copy(vsb[:, j, 0:D], pv[:, 64:128])

            def kv_finish(g, p):
                cols = slice(g * 512, (g + 1) * 512)
                nc.vector.tensor_scalar(kv_sb[:, cols], p[:], bkv, None, ADD)

            def kv_group(g):
                cols = slice(g * 512, (g + 1) * 512)
                p = ps_p.tile([128, 512], f32, tag="pp")
                for e in range(NE):
                    nc.tensor.matmul(
                        p[:],
                        wkv[:, e * 128 : (e + 1) * 128],
                        xt[e][:, cols],
                        start=(e == 0),
                        stop=(e == NE - 1),
                    )
                    yield
                kv_finish(g, p)
                for j in range(4 * g, 4 * g + 4):
                    yield from vt_one(j)

            def q_group(g):
                cols = slice(g * 512, (g + 1) * 512)
                p = ps_q.tile([D, 512], f32, tag="ppq")
                for e in range(NE):
                    nc.tensor.matmul(
                        p[:],
                        wq[:, e * D : (e + 1) * D],
                        xt[e][:, cols],
                        start=(e == 0),
                        stop=(e == NE - 1),
                    )
                    yield
                nc.vector.tensor_scalar(qown[:, cols], p[:], bq, None, ADD)

            def mk_state(jlist, spec, qbase, pool, tag):
                return {"jlist": jlist, "spec": spec, "qbase": qbase,
                        "pool": pool, "tag": tag, "pts": {}}

            def emit_score(st, idx):
                nz, mt = st["spec"][idx]
                j = st["jlist"][idx]
                qbase = st["qbase"]
                qc = slice(nz * 128, 512)
                psc = st["pool"].tile([128, 512], f32, tag=st["tag"])
                nc.tensor.matmul(
                    psc[:, qc],
                    kv_sb[0:D, j * 128 : (j + 1) * 128],
                    qown[:, qbase + nz * 128 : qbase + 512],
                    start=True,
                    stop=True,
                )
                pt = probs.tile([128, 512], bf16, tag="pt")
                nc.scalar.activation(pt[:, qc], psc[:, qc], EXP)
                if mt is not None:
                    mc = slice(nz * 128, (nz + 1) * 128)
                    nc.vector.tensor_mul(pt[:, mc], pt[:, mc], mt)
                st["pts"][idx] = (pt, qc)

            def attn_scores(st, idxs):
                for idx in idxs:
                    emit_score(st, idx)
                    yield

            def attn(st, pout, slot_base, epi_from, pre=()):
                # PVs for all idx; scores emitted one ahead, except those in
                # `pre` which were pre-emitted elsewhere (attn_scores)
                jlist, pts = st["jlist"], st["pts"]
                n = len(jlist)
                depth = 2 if dmt else 1
                for i in range(min(depth, n)):
                    if i not in pre:
                        emit_score(st, i)
                        yield
                for idx in range(n):
                    t = idx + depth
                    if t < n and t not in pre:
                        emit_score(st, t)
                        yield
                    pt, qc = pts.pop(idx)
                    # skip_group_check: the streamed epilogue reads finished
                    # sub-block columns while the bank's accumulation group is
                    # still open for higher columns (fine on HW, sim-only
                    # check)
                    nc.tensor.matmul(
                        pout[:, qc],
                        vsb[:, jlist[idx], :],
                        pt[:, qc],
                        start=(idx == 0),
                        stop=(idx == n - 1),
                        skip_group_check=True,
                    )
                    yield
                    # stream the epilogue: sub-block s is final after the PV
                    # at idx == epi_from + s
                    s = idx - epi_from
                    if 0 <= s < 4:
                        if dmt:
                            # bf16 copy (rows 0:65 of an 80-row tile so the
                            # xbar sees whole 16-row tiles; rows 65:80 are
                            # garbage that lands in unread dst columns)
                            ot = smalls.tile([80, 128], bf16, tag="otT")
                            nc.gpsimd.memset(ot[D : 80, :], 0.0)
                            nc.vector.tensor_copy(
                                ot[0 : D + 1, :], pout[:, s * 128 : (s + 1) * 128]
                            )
                            nat = smalls.tile([128, 80], bf16, tag="nat")
                            nc.sync.dma_start_transpose(nat[:], ot[:])
                            rcp = smalls.tile([128, 1], f32, tag="rcp")
                            nc.vector.reciprocal(rcp[:], nat[:, D : D + 1])
                            fin = smalls.tile([128, D], f32, tag="fin")
                            nc.vector.tensor_scalar(
                                fin[:], nat[:, 0:D], rcp[:], None, MUL
                            )
                            nc.sync.dma_start(out_d[slot_base + s], fin[:])
                            continue
                        ot = smalls.tile([D + 1, 128], bf16, tag="otT")
                        nc.vector.tensor_copy(
                            ot[:], pout[:, s * 128 : (s + 1) * 128]
                        )
                        ptr = ps_t.tile([128, 130], bf16, tag="ps_vt")
                        nc.tensor.transpose(
                            ptr[:, 0 : D + 1], ot[:], idb[0 : D + 1, 0 : D + 1]
                        )
                        yield
                        rcp = smalls.tile([128, 1], f32, tag="rcp")
                        nc.vector.reciprocal(rcp[:], ptr[:, D : D + 1])
                        fin = smalls.tile([128, D], f32, tag="fin")
                        nc.vector.tensor_scalar(
                            fin[:], ptr[:, 0:D], rcp[:], None, MUL
                        )
                        nc.sync.dma_start(out_d[slot_base + s], fin[:])

            # group A: key positions 0..3 (own-parity, tri on diag) and 8..11
            # (other-parity, mab on boundary); below-diagonal sub-blocks are
            # skipped via shrinking-N.
            spec_a = [(i, tri) for i in range(4)] + [(i, mab) for i in range(4)]
            # group B: owned blocks 4..7 vs all 16 key positions.
            spec_b = []
            for j in range(16):
                if 4 <= j < 8:
                    spec_b.append((j - 4, tri))
                elif 12 <= j:
                    spec_b.append((j - 12, mab))
                else:
                    spec_b.append((0, None))

            def run(gen):
                for _ in gen:
                    pass

            # ---- phase 1: KV g0, KV g2 and Q accumulate together, chunk by
            # chunk, tracking x DMA arrival
            pg0 = ps_p.tile([128, 512], f32, tag="pp")
            pg2 = ps_p.tile([128, 512], f32, tag="pp")
            qa = q_group(0)
            for e in range(NE):
                nc.tensor.matmul(
                    pg0[:], wkv[:, e * 128 : (e + 1) * 128], xt[e][:, 0:512],
                    start=(e == 0), stop=(e == NE - 1),
                )
                nc.tensor.matmul(
                    pg2[:], wkv[:, e * 128 : (e + 1) * 128], xt[e][:, 1024:1536],
                    start=(e == 0), stop=(e == NE - 1),
                )
                next(qa, None)
            next(qa, None)  # emit the q bias-add
            kv_finish(0, pg0)
            kv_finish(2, pg2)
            for j in JLIST_A:
                run(vt_one(j))

            # ---- phase 2: attn A, interleaving the remaining projections
            stA = mk_state(JLIST_A, spec_a, 0, ps_sc, "psc")
            stB = mk_state(list(range(16)), spec_b, 512, ps_p, "pp")
            poutA = ps_o.tile([D + 1, 512], f32, tag="pout")
            # attn B's first 4 (maskless) scores+exps ride in phase 2 where
            # the scalar engine has slack; their PVs run in phase 3
            fillers = itertools.chain(
                q_group(1),
                attn_scores(stB, range(0, 4)),
                kv_group(1),
                attn_scores(stB, range(8, 12)),
                kv_group(3),
            )
            for _ in attn(stA, poutA, 0, 4):
                next(fillers, None)
                next(fillers, None)
            for _ in fillers:
                pass

            # ---- phase 3: attn B with streamed epilogue
            poutB = ps_o.tile([D + 1, 512], f32, tag="pout")
            run(attn(stB, poutB, 4, 12, pre=frozenset((0, 1, 2, 3, 8, 9, 10, 11))))

    nc.compile()
    return nc


def _get_nc():
    key = os.environ.get("KVARIANT", "full")
    if key not in _BUILT:
        _BUILT[key] = _build()
    return _BUILT[key]


def _host_inputs(x, Wq, bq, Wk, bk, Wv, bv):
    """Build the 8 per-core input maps."""
    import ml_dtypes

    bf = ml_dtypes.bfloat16
    x = np.asarray(x, np.float32)
    cbf0 = np.zeros((128, CBF_N), np.float32)
    cbf0[:, CBF_WKV : CBF_WKV + NE * 128] = (
        np.concatenate(
            [
                np.asarray(Wk, np.float32).reshape(NE, 128, D),
                np.asarray(Wv, np.float32).reshape(NE, 128, D),
            ],
            axis=2,
        )
        .transpose(1, 0, 2)
        .reshape(128, NE * 128)
    )
    cbf0[:, CBF_WQ : CBF_WQ + NE * D] = (
        (np.asarray(Wq, np.float32) / float(D))
        .reshape(NE, 128, D)
        .transpose(1, 0, 2)
        .reshape(128, NE * D)
    )
    cbf0[:, CBF_TRI : CBF_TRI + 128] = np.triu(np.ones((128, 128), np.float32))
    cbf0[:, CBF_IDB : CBF_IDB + 128] = np.eye(128, dtype=np.float32)

    cf = np.zeros((128, CF_N), np.float32)
    cf[:, CF_BKV] = np.concatenate(
        [np.asarray(bk, np.float32), np.asarray(bv, np.float32)]
    )
    cf[0:D, CF_BQ] = np.asarray(bq, np.float32) / float(D)

    xbT = [np.ascontiguousarray(x[b].T) for b in range(B)]  # [E, S]
    in_maps = []
    for c in range(NCORES):
        b, h = c // 2, c % 2
        perm = [2 * p + (1 - h) for p in range(8)] + [2 * p + h for p in range(8)]
        xp = xbT[b].reshape(E, NB, 128)[:, perm, :].reshape(E, S)
        xT = (
            xp.reshape(NE, 128, S).transpose(1, 0, 2).reshape(128, NE * S).astype(bf)
        )
        cbf = cbf0.copy()
        cbf[:, CBF_MAB : CBF_MAB + 128] = 1.0 - h
        in_maps.append({
            "xT": xT,
            "cbf": cbf.astype(bf),
            "cf": cf,
        })
    return in_maps


def _assemble(results):
    out = np.zeros((B, S, D), np.float32)
    for c in range(NCORES):
        b, h = c // 2, c % 2
        o = np.asarray(results[c]["out"]).reshape(NSLOT, 128, D)
        for i in range(NSLOT):
            g = 2 * i + (1 - h)
            out[b, g * 128 : (g + 1) * 128] = o[i]
    return out


def kernel(x, Wq, bq, Wk, bk, Wv, bv):
    global LAST
    from concourse.bass_utils import run_bass_kernel_spmd

    nc = _get_nc()
    in_maps = _host_inputs(x, Wq, bq, Wk, bk, Wv, bv)
    LAST = run_bass_kernel_spmd(nc, in_maps, list(range(NCORES)))
    return _assemble(LAST.results)


# revision 47
# speedup vs baseline: 1.1371x; 1.0069x over previous
"""Causal single-head attention (B=4, S=2048, E=1024, D=64) on 8 TRN2 NeuronCores.

Sharding: core c -> batch b = c//2, parity h = c%2. Owned query blocks are the
8 128-token blocks of parity (1-h) (h=0 -> odd, h=1 -> even), balancing causal
work 68/136 per core. No collectives: each core projects full K/V for its batch.

Perf structure: per-matmul fixed overhead (~190ns) dominates small-N
instructions, so everything is restructured into few, wide (N>=512) matmuls,
and the schedule is arranged so the PE never waits:
  - x columns are PERMUTED on the host to [owned blocks asc | other blocks asc]
    so the owned-Q projection is contiguous and scores/PV address key blocks by
    position with a graph identical across cores (SPMD); all per-core variation
    is in input data (x permutation, mab mask).
  - K and V projections are PACKED into one M=128 matmul per (col group,
    E-chunk): lhsT = [Wk_e | Wv_e], psum rows 0:64 = K^T, 64:128 = V^T.
  - phase 1 interleaves three accumulations (KV g0, KV g2, Q over both halves)
    per x chunk so compute tracks DMA arrival; constants ride in a blob DMA;
    x chunks are spread over three DMA issue queues (sync/scalar/gpsimd).
  - attention runs in 2 super-groups of 4 query blocks: group A (owned 0..3)
    over 8 key positions, group B (owned 4..7) over all 16. Below-diagonal
    query sub-blocks are not computed (shrinking-N), the boundary sub-block
    gets one [128,128] mask multiply (tri for own-parity keys, all-0/1 "mab"
    for other-parity keys). Score pairs share one bf16 psum tile and ONE exp
    instruction; pairs run one step ahead of PV so exp latency is hidden.
    Group A overlaps the remaining projections; both groups stream their
    epilogue per sub-block as soon as that sub-block's accumulation finishes.
  - scoresT layout [k, q] everywhere; PV accumulates out^T[65, q] f32 with a
    ones column appended to V (row 64 = softmax denominator); bf16 PE
    transpose + reciprocal-multiply normalizes.
"""

import itertools
import os
import sys

sys.path.insert(0, "/opt/trn_rl_repo")

import numpy as np

B, S, E, D = 4, 2048, 1024, 64
NB = S // 128      # 16 token blocks
NSLOT = NB // 2    # 8 owned query blocks per core
NE = E // 128      # 8 contraction chunks
NCORES = 8

JLIST_A = [0, 1, 2, 3, 8, 9, 10, 11]
# const blob layout (columns, bf16)
CBF_WKV, CBF_WQ, CBF_TRI, CBF_MAB, CBF_IDB = 0, 1024, 1536, 1664, 1792
CBF_N = 1920
CF_BKV, CF_BQ = 0, 1
CF_N = 2

_BUILT = {}
LAST = None  # BassKernelResults of the most recent run (for test harness)


def _build():
    variant = os.environ.get("KVARIANT", "full")
    from concourse import bacc, bass, tile, mybir

    f32 = mybir.dt.float32
    bf16 = mybir.dt.bfloat16
    MUL = mybir.AluOpType.mult
    ADD = mybir.AluOpType.add
    EXP = mybir.ActivationFunctionType.Exp

    # xbar DMA transposes verify in isolation but race with their consumers
    # under load on HW (inf outputs); keep PE transposes unless experimenting.
    dmt = variant == "dmt"

    nc = bacc.Bacc(None, target_bir_lowering=False, debug=False)

    xT_d = nc.declare_dram_parameter("xT", [128, NE * S], bf16, isOutput=False)
    cbf_d = nc.declare_dram_parameter("cbf", [128, CBF_N], bf16, isOutput=False)
    cf_d = nc.declare_dram_parameter("cf", [128, CF_N], f32, isOutput=False)
    out_d = nc.declare_dram_parameter("out", [NSLOT, 128, D], f32, isOutput=True)

    with tile.TileContext(nc) as tc:
        with (
            tc.tile_pool(name="consts", bufs=1) as consts,
            tc.tile_pool(name="xpool", bufs=NE) as xpool,
            tc.tile_pool(name="acts", bufs=1) as acts,
            tc.tile_pool(name="probs", bufs=18) as probs,
            tc.tile_pool(name="smalls", bufs=3) as smalls,
            tc.tile_pool(name="ps_p", bufs=2, space="PSUM") as ps_p,
            tc.tile_pool(name="ps_q", bufs=1, space="PSUM") as ps_q,
            tc.tile_pool(name="ps_sc", bufs=3 if dmt else 2, space="PSUM") as ps_sc,
            tc.tile_pool(name="ps_o", bufs=2 if dmt else 1, space="PSUM") as ps_o,
            tc.tile_pool(name="ps_t", bufs=2, space="PSUM") as ps_t,
            # banks (dmt): ps_p 2 + ps_q 1 + ps_sc 3 + ps_o 2 = 8 (ps_t unused)
            # banks (pet): ps_p 2 + ps_q 1 + ps_sc 2 + ps_o 1 + ps_t 2 = 8
        ):
            # ---- constants: blob DMAs
            cbf = consts.tile([128, CBF_N], bf16, tag="cbf")
            cf = consts.tile([128, CF_N], f32, tag="cf")
            nc.sync.dma_start(cbf[:], cbf_d[:])
            nc.scalar.dma_start(cf[:], cf_d[:])
            wkv = cbf[:, CBF_WKV : CBF_WKV + NE * 128]
            wq = cbf[:, CBF_WQ : CBF_WQ + NE * D]
            tri = cbf[:, CBF_TRI : CBF_TRI + 128]
            mab = cbf[:, CBF_MAB : CBF_MAB + 128]
            idb = cbf[:, CBF_IDB : CBF_IDB + 128]
            bkv = cf[:, CF_BKV : CF_BKV + 1]
            bq = cf[0:D, CF_BQ : CF_BQ + 1]

            # ---- x^T tiles (permuted cols), spread over three issue queues
            xt = []
            qeng = [nc.sync, nc.scalar, nc.gpsimd, nc.sync,
                    nc.scalar, nc.gpsimd, nc.sync, nc.scalar]
            for e in range(NE):
                t = xpool.tile([128, S], bf16, tag="xt")
                qeng[e].dma_start(t[:], xT_d[:, e * S : (e + 1) * S])
                xt.append(t)

            # ---- persistent activations
            kv_sb = acts.tile([128, S], bf16, tag="kv")     # 0:64 K^T, 64:128 V^T
            qown = acts.tile([D, NSLOT * 128], bf16, tag="qown")
            vsb = acts.tile([128, NB, D + 1], bf16, tag="vsb")
            nc.vector.memset(vsb[:, :, D : D + 1], 1.0)

            def vt_one(j):
                # V natural block j: transpose the V rows of the packed K/V
                # slice (xbar DMA, or bf16 PE transpose as fallback).
                if dmt:
                    nc.sync.dma_start_transpose(
                        vsb[:, j, 0:D], kv_sb[D:128, j * 128 : (j + 1) * 128]
                    )
                    return
                pv = ps_t.tile([128, 130], bf16, tag="ps_vt")
                nc.tensor.transpose(
                    pv[:, 0:128], kv_sb[:, j * 128 : (j + 1) * 128], idb
                )
                yield
                nc.vector.tensor_copy(vsb[:, j, 0:D], pv[:, 64:128])

            def kv_finish(g, p):
                cols = slice(g * 512, (g + 1) * 512)
                nc.vector.tensor_scalar(kv_sb[:, cols], p[:], bkv, None, ADD)

            def kv_group(g):
                cols = slice(g * 512, (g + 1) * 512)
                p = ps_p.tile([128, 512], f32, tag="pp")
                for e in range(NE):
                    nc.tensor.matmul(
                        p[:],
                        wkv[:, e * 128 : (e + 1) * 128],
                        xt[e][:, cols],
                        start=(e == 0),
                        stop=(e == NE - 1),
                    )
                    yield
                kv_finish(g, p)
                for j in range(4 * g, 4 * g + 4):
                    yield from vt_one(j)

            def q_group(g):
                cols = slice(g * 512, (g + 1) * 512)
                p = ps_q.tile([D, 512], f32, tag="ppq")
                for e in range(NE):
                    nc.tensor.matmul(
                        p[:],
                        wq[:, e * D : (e + 1) * D],
                        xt[e][:, cols],
                        start=(e == 0),
                        stop=(e == NE - 1),
                    )
                    yield
                nc.vector.tensor_scalar(qown[:, cols], p[:], bq, None, ADD)

            def mk_state(jlist, spec, qbase, pool, tag):
                return {"jlist": jlist, "spec": spec, "qbase": qbase,
                        "pool": pool, "tag": tag, "pts": {}}

            def emit_score(st, idx):
                nz, mt = st["spec"][idx]
                j = st["jlist"][idx]
                qbase = st["qbase"]
                qc = slice(nz * 128, 512)
                psc = st["pool"].tile([128, 512], f32, tag=st["tag"])
                nc.tensor.matmul(
                    psc[:, qc],
                    kv_sb[0:D, j * 128 : (j + 1) * 128],
                    qown[:, qbase + nz * 128 : qbase + 512],
                    start=True,
                    stop=True,
                )
                pt = probs.tile([128, 512], bf16, tag="pt")
                nc.scalar.activation(pt[:, qc], psc[:, qc], EXP)
                if mt is not None:
                    mc = slice(nz * 128, (nz + 1) * 128)
                    nc.vector.tensor_mul(pt[:, mc], pt[:, mc], mt)
                st["pts"][idx] = (pt, qc)

            def attn_scores(st, idxs):
                for idx in idxs:
                    emit_score(st, idx)
                    yield

            def attn(st, pout, slot_base, epi_from, pre=()):
                # PVs for all idx; scores emitted one ahead, except those in
                # `pre` which were pre-emitted elsewhere (attn_scores)
                jlist, pts = st["jlist"], st["pts"]
                n = len(jlist)
                depth = 2 if dmt else 1
                for i in range(min(depth, n)):
                    if i not in pre:
                        emit_score(st, i)
                        yield
                for idx in range(n):
                    t = idx + depth
                    if t < n and t not in pre:
                        emit_score(st, t)
                        yield
                    pt, qc = pts.pop(idx)
                    # skip_group_check: the streamed epilogue reads finished
                    # sub-block columns while the bank's accumulation group is
                    # still open for higher columns (fine on HW, sim-only
                    # check)
                    nc.tensor.matmul(
                        pout[:, qc],
                        vsb[:, jlist[idx], :],
                        pt[:, qc],
                        start=(idx == 0),
                        stop=(idx == n - 1),
                        skip_group_check=True,
                    )
                    yield
                    # stream the epilogue: sub-block s is final after the PV
                    # at idx == epi_from + s
                    s = idx - epi_from
                    if 0 <= s < 4:
                        if dmt:
                            # bf16 copy (rows 0:65 of an 80-row tile so the
                            # xbar sees whole 16-row tiles; rows 65:80 are
                            # garbage that lands in unread dst columns)
                            ot = smalls.tile([80, 128], bf16, tag="otT")
                            nc.gpsimd.memset(ot[D : 80, :], 0.0)
                            nc.vector.tensor_copy(
                                ot[0 : D + 1, :], pout[:, s * 128 : (s + 1) * 128]
                            )
                            nat = smalls.tile([128, 80], bf16, tag="nat")
                            nc.sync.dma_start_transpose(nat[:], ot[:])
                            rcp = smalls.tile([128, 1], f32, tag="rcp")
                            nc.vector.reciprocal(rcp[:], nat[:, D : D + 1])
                            fin = smalls.tile([128, D], f32, tag="fin")
                            nc.vector.tensor_scalar(
                                fin[:], nat[:, 0:D], rcp[:], None, MUL
                            )
                            nc.sync.dma_start(out_d[slot_base + s], fin[:])
                            continue
                        ot = smalls.tile([D + 1, 128], bf16, tag="otT")
                        nc.vector.tensor_copy(
                            ot[:], pout[:, s * 128 : (s + 1) * 128]
                        )
                        ptr = ps_t.tile([128, 130], bf16, tag="ps_vt")
                        nc.tensor.transpose(
                            ptr[:, 0 : D + 1], ot[:], idb[0 : D + 1, 0 : D + 1]
                        )
                        yield
                        rcp = smalls.tile([128, 1], f32, tag="rcp")
                        nc.vector.reciprocal(rcp[:], ptr[:, D : D + 1])
                        fin = smalls.tile([128, D], f32, tag="fin")
                        nc.vector.tensor_scalar(
                            fin[:], ptr[:, 0:D], rcp[:], None, MUL
                        )
                        nc.sync.dma_start(out_d[slot_base + s], fin[:])

            # group A: key positions 0..3 (own-parity, tri on diag) and 8..11
            # (other-parity, mab on boundary); below-diagonal sub-blocks are
            # skipped via shrinking-N.
            spec_a = [(i, tri) for i in range(4)] + [(i, mab) for i in range(4)]
            # group B: owned blocks 4..7 vs all 16 key positions.
            spec_b = []
            for j in range(16):
                if 4 <= j < 8:
                    spec_b.append((j - 4, tri))
                elif 12 <= j:
                    spec_b.append((j - 12, mab))
                else:
                    spec_b.append((0, None))

            def run(gen):
                for _ in gen:
                    pass

            # ---- phase 1: KV g0, KV g2 and Q accumulate together, chunk by
            # chunk, tracking x DMA arrival
            pg0 = ps_p.tile([128, 512], f32, tag="pp")
            pg2 = ps_p.tile([128, 512], f32, tag="pp")
            qa = q_group(0)
            for e in range(NE):
                nc.tensor.matmul(
                    pg0[:], wkv[:, e * 128 : (e + 1) * 128], xt[e][:, 0:512],
                    start=(e == 0), stop=(e == NE - 1),
                )
                nc.tensor.matmul(
                    pg2[:], wkv[:, e * 128 : (e + 1) * 128], xt[e][:, 1024:1536],
                    start=(e == 0), stop=(e == NE - 1),
                )
                next(qa, None)
            next(qa, None)  # emit the q bias-add
            kv_finish(0, pg0)
            kv_finish(2, pg2)
            for j in JLIST_A:
                run(vt_one(j))

            # ---- phase 2: attn A, interleaving the remaining projections
            stA = mk_state(JLIST_A, spec_a, 0, ps_sc, "psc")
            stB = mk_state(list(range(16)), spec_b, 512, ps_p, "pp")
            poutA = ps_o.tile([D + 1, 512], f32, tag="pout")
            # attn B's first 4 (maskless) scores+exps ride in phase 2 where
            # the scalar engine has slack; their PVs run in phase 3
            fillers = itertools.chain(
                q_group(1),
                attn_scores(stB, range(0, 4)),
                kv_group(1),
                attn_scores(stB, range(8, 12)),
                kv_group(3),
                attn_scores(stB, range(4, 8)),
                attn_scores(stB, range(12, 16)),
            )
            for _ in attn(stA, poutA, 0, 4):
                next(fillers, None)
                next(fillers, None)
            for _ in fillers:
                pass

            # ---- phase 3: attn B with streamed epilogue
            poutB = ps_o.tile([D + 1, 512], f32, tag="pout")
            run(attn(stB, poutB, 4, 12, pre=frozenset(range(16))))

    nc.compile()
    return nc


def _get_nc():
    key = os.environ.get("KVARIANT", "full")
    if key not in _BUILT:
        _BUILT[key] = _build()
    return _BUILT[key]


def _host_inputs(x, Wq, bq, Wk, bk, Wv, bv):
    """Build the 8 per-core input maps."""
    import ml_dtypes

    bf = ml_dtypes.bfloat16
    x = np.asarray(x, np.float32)
    cbf0 = np.zeros((128, CBF_N), np.float32)
    cbf0[:, CBF_WKV : CBF_WKV + NE * 128] = (
        np.concatenate(
            [
                np.asarray(Wk, np.float32).reshape(NE, 128, D),
                np.asarray(Wv, np.float32).reshape(NE, 128, D),
            ],
            axis=2,
        )
        .transpose(1, 0, 2)
        .reshape(128, NE * 128)
    )
    cbf0[:, CBF_WQ : CBF_WQ + NE * D] = (
        (np.asarray(Wq, np.float32) / float(D))
        .reshape(NE, 128, D)
        .transpose(1, 0, 2)
        .reshape(128, NE * D)
    )
    cbf0[:, CBF_TRI : CBF_TRI + 128] = np.triu(np.ones((128, 128), np.float32))
    cbf0[:, CBF_IDB : CBF_IDB + 128] = np.eye(128, dtype=np.float32)

    cf = np.zeros((128, CF_N), np.float32)
    cf[:, CF_BKV] = np.concatenate(
        [np.asarray(bk, np.float32), np.asarray(bv, np.float32)]
    )
    cf[0:D, CF_BQ] = np.asarray(bq, np.float32) / float(D)

    xbT = [np.ascontiguousarray(x[b].T) for b in range(B)]  # [E, S]
    in_maps = []
    for c in range(NCORES):
        b, h = c // 2, c % 2
        perm = [2 * p + (1 - h) for p in range(8)] + [2 * p + h for p in range(8)]
        xp = xbT[b].reshape(E, NB, 128)[:, perm, :].reshape(E, S)
        xT = (
            xp.reshape(NE, 128, S).transpose(1, 0, 2).reshape(128, NE * S).astype(bf)
        )
        cbf = cbf0.copy()
        cbf[:, CBF_MAB : CBF_MAB + 128] = 1.0 - h
        in_maps.append({
            "xT": xT,
            "cbf": cbf.astype(bf),
            "cf": cf,
        })
    return in_maps


def _assemble(results):
    out = np.zeros((B, S, D), np.float32)
    for c in range(NCORES):
        b, h = c // 2, c % 2
        o = np.asarray(results[c]["out"]).reshape(NSLOT, 128, D)
        for i in range(NSLOT):
            g = 2 * i + (1 - h)
            out[b, g * 128 : (g + 1) * 128] = o[i]
    return out


def kernel(x, Wq, bq, Wk, bk, Wv, bv):
    global LAST
    from concourse.bass_utils import run_bass_kernel_spmd

    nc = _get_nc()
    in_maps = _host_inputs(x, Wq, bq, Wk, bk, Wv, bv)
    LAST = run_bass_kernel_spmd(nc, in_maps, list(range(NCORES)))
    return _assemble(LAST.results)
